# revision 10
# baseline (speedup 1.0000x reference)
"""CIN (Compressed Interaction Network) kernel for Trainium2, 8 NeuronCores.

Reference computation (per batch b, embedding dim d; x has 40 field vectors):
  h0[s] = relu( sum_{i,j} x_i x_j W0[i,j,s] + b0[s] )          s in 0..128
  nh    = h0[0:64];  d0 = h0[64:128]
  h1[s] = relu( sum_{i<40, j<64} x_i nh_j W1[i,j,s] + b1[s] )
  out   = concat(d0, h1, axis=s) summed over d                 -> (B, 192)

Strategy
--------
Pure data parallel over the batch (B=2048 -> 256 per core).  Per (b, d)
pair ("bd" column, 8192 per core) the two contractions are quadratic /
bilinear forms over small vectors.  Using the polarization identity
  a*b = ((a+b)^2 - a^2 - b^2) / 2
each layer becomes:   project (PE matmul, K-major)  ->  square
(ACT/DVE elementwise, PSUM->SBUF)  ->  contract (PE matmul, accumulate).
This avoids all transposes and partition broadcasts; every matmul is
K-major with bd on the free axis.  All matmul operands are bf16 (fp32
accumulation in PSUM); coefficient matrices are precomputed on host.

Layer 0: features = 780 pair sums (x_i+x_j, i<j) + 40 singles (x_i),
padded to 896 = 7*128.  Layer 1: 2560 pair sums (x_i + nh_j) in 20
chunks + one K=104 contraction against squared [x; nh] singles.
"""

import numpy as np
import ml_dtypes

B, F0, D = 2048, 40, 32
NCORES = 8
BC = B // NCORES       # 256 batches per core
BD = BC * D            # 8192 bd columns per core
NHF = 64               # next-hidden fields (split_half of 128)
S0 = 128               # layer 0 outputs
S1 = 128               # layer 1 outputs
KU = F0 + NHF          # 104

PAIRS0 = [(i, j) for i in range(F0) for j in range(i + 1, F0)]   # 780
R0 = len(PAIRS0) + F0   # 820
NC0 = 7                 # L0 feature chunks (896 padded)
R0P = NC0 * 128
R1 = F0 * NHF           # 2560 L1 pair features
NC1 = R1 // 128         # 20

NFREE = 512             # bd columns per pipeline chunk
NMM = 512               # max free dim per matmul instruction
NCHUNKS = BD // NFREE
NBPC = NFREE // D       # batches per chunk

BF16 = ml_dtypes.bfloat16

_cached = {}


def _build_host_weights(W0, b0, W1, b1):
    """Precompute projection/coefficient matrices (float64 for the
    cancellation-prone single coefficients, cast to bf16 at the end)."""
    W0 = np.asarray(W0, np.float64)
    W1 = np.asarray(W1, np.float64)

    # ---- layer 0 ----
    p0 = np.zeros((F0, R0P), np.float64)
    c0 = np.zeros((R0P, S0), np.float64)
    Ssym = (W0 + W0.transpose(1, 0, 2)) / 2.0          # [i, j, s]
    for k, (i, j) in enumerate(PAIRS0):
        p0[i, k] = 1.0
        p0[j, k] = 1.0
        c0[k] = Ssym[i, j]
    for i in range(F0):
        k = len(PAIRS0) + i
        p0[i, k] = 1.0
        c0[k] = W0[i, i] - (Ssym[i].sum(axis=0) - Ssym[i, i])
    # chunk layout: feature f lives at partition f%128, free col (f//128)*S0 + s
    c0_sb = c0.reshape(NC0, 128, S0).transpose(1, 0, 2).reshape(128, NC0 * S0)

    # ---- layer 1 ----
    # U layout (SBUF partition rows must start 32-aligned): nh_j at row j
    # (0:64), x_i at row 64+i (64:104).
    p1 = np.zeros((KU, R1), np.float64)
    c1p = np.zeros((R1, S1), np.float64)
    for i in range(F0):
        for j in range(NHF):
            k = i * NHF + j
            p1[NHF + i, k] = 1.0
            p1[j, k] = 1.0
            c1p[k] = W1[i, j] / 2.0
    c1p_sb = c1p.reshape(NC1, 128, S1).transpose(1, 0, 2).reshape(128, NC1 * S1)
    c1s = np.zeros((KU, S1), np.float64)
    c1s[:NHF] = -0.5 * W1.sum(axis=0)                  # vs nh_j^2
    c1s[NHF:] = -0.5 * W1.sum(axis=1)                  # vs x_i^2

    return {
        "p0": p0.astype(BF16),
        "c0": c0_sb.astype(BF16),
        "p1": p1.astype(BF16),
        "c1p": c1p_sb.astype(BF16),
        "c1s": c1s.astype(BF16),
        "b0": np.asarray(b0, np.float32).reshape(S0, 1),
        "b1": np.asarray(b1, np.float32).reshape(S1, 1),
    }


def _split_multi_waits(nc):
    """The walrus build in this container rejects any instruction carrying
    more than one sync wait ("Too many sync wait commands").  Hoist all but
    one wait of every multi-wait instruction onto same-engine NOPs placed
    immediately before it (engines execute their stream in order, so this
    preserves the happens-before edges)."""
    import concourse.mybir as mybir

    n = 0
    for blk in nc.main_func.blocks:
        insts = blk.instructions
        out = []
        changed = False
        for inst in insts:
            si = getattr(inst, "sync_info", None)
            if si is not None and si.on_wait and len(si.on_wait) > 1:
                waits = list(si.on_wait)
                for w in waits[:-1]:
                    nop = mybir.InstNoOp(
                        name=f"waitsplit_{n}",
                        engine=inst.engine,
                        sync_info=mybir.SyncInfo(on_wait=[w], on_update=[]),
                        bass_nofuse=True,
                    )
                    n += 1
                    out.append(nop)
                si.on_wait = waits[-1:]
                changed = True
            out.append(inst)
        if changed:
            blk.instructions = out
    return n


def _build_nc():
    import concourse.bass as bass
    import concourse.tile as tile
    import concourse.mybir as mybir

    dt = mybir.dt
    AF = mybir.ActivationFunctionType
    ALU = mybir.AluOpType

    nc = bass.Bass()

    NXT = 4                       # x input split into NXT tiles for DMA overlap
    XTW = BD // NXT               # 2048 cols each
    xt_d = [
        nc.dram_tensor(f"xt{t}", [F0, XTW], dt.bfloat16, kind="ExternalInput")
        for t in range(NXT)
    ]
    p0_d = nc.dram_tensor("p0", [F0, R0P], dt.bfloat16, kind="ExternalInput")
    c0_d = nc.dram_tensor("c0", [128, NC0 * S0], dt.bfloat16, kind="ExternalInput")
    p1_d = nc.dram_tensor("p1", [KU, R1], dt.bfloat16, kind="ExternalInput")
    c1p_d = nc.dram_tensor("c1p", [128, NC1 * S1], dt.bfloat16, kind="ExternalInput")
    c1s_d = nc.dram_tensor("c1s", [KU, S1], dt.bfloat16, kind="ExternalInput")
    b0_d = nc.dram_tensor("b0", [S0, 1], dt.float32, kind="ExternalInput")
    b1_d = nc.dram_tensor("b1", [S1, 1], dt.float32, kind="ExternalInput")
    out_d = nc.dram_tensor("out", [S0 - NHF + S1, BC], dt.float32,
                           kind="ExternalOutput")

    with tile.TileContext(nc) as tc:
        with (
            tc.tile_pool(name="const", bufs=1) as const_pool,
            tc.tile_pool(name="xt", bufs=1) as xt_pool,
            tc.tile_pool(name="sq", bufs=8) as sq_pool,
            tc.tile_pool(name="u", bufs=3) as u_pool,
            tc.tile_pool(name="d", bufs=3) as d_pool,
            tc.tile_pool(name="outp", bufs=1) as out_pool,
            tc.tile_pool(name="vps", bufs=4, space="PSUM") as vps_pool,
            tc.tile_pool(name="h0ps", bufs=2, space="PSUM") as h0_pool,
            tc.tile_pool(name="h1ps", bufs=2, space="PSUM") as h1_pool,
        ):
            p0_sb = const_pool.tile([F0, R0P], dt.bfloat16)
            c0_sb = const_pool.tile([128, NC0 * S0], dt.bfloat16)
            p1_sb = const_pool.tile([KU, R1], dt.bfloat16)
            c1p_sb = const_pool.tile([128, NC1 * S1], dt.bfloat16)
            c1s_sb = const_pool.tile([KU, S1], dt.bfloat16)
            b0_sb = const_pool.tile([S0, 1], dt.float32)
            b1_sb = const_pool.tile([S1, 1], dt.float32)
            nc.gpsimd.dma_start(out=p0_sb[:], in_=p0_d[:])
            nc.gpsimd.dma_start(out=c0_sb[:], in_=c0_d[:])
            nc.gpsimd.dma_start(out=p1_sb[:], in_=p1_d[:])
            nc.gpsimd.dma_start(out=c1p_sb[:], in_=c1p_d[:])
            nc.gpsimd.dma_start(out=c1s_sb[:], in_=c1s_d[:])
            nc.gpsimd.dma_start(out=b0_sb[:], in_=b0_d[:])
            nc.gpsimd.dma_start(out=b1_sb[:], in_=b1_d[:])

            xt_sb = []
            for t in range(NXT):
                xt = xt_pool.tile([F0, XTW], dt.bfloat16, tag=f"xt{t}")
                nc.gpsimd.dma_start(out=xt[:], in_=xt_d[t][:])
                xt_sb.append(xt)

            out0_sb = out_pool.tile([S0 - NHF, BC], dt.float32, tag="o0")
            out1_sb = out_pool.tile([S1, BC], dt.float32, tag="o1")

            def square(idx, dst, src, pool):
                # PSUM evacuation split: ACT squares directly; the rest go
                # DVE copy (PSUM->SBUF bf16) + GpSimd multiply (SBUF bf16).
                # DVE cannot read the PSUM operand twice (NCC_IBVF027).
                if idx % 9 < 5:
                    nc.scalar.square(dst, src)
                else:
                    tmp = pool.tile(list(dst.shape), dst.dtype, tag="sqtmp")
                    nc.vector.tensor_copy(out=tmp[:], in_=src)
                    nc.gpsimd.tensor_mul(dst, tmp[:], tmp[:])

            for ch in range(NCHUNKS):
                xt = xt_sb[(ch * NFREE) // XTW]
                xcols = slice((ch * NFREE) % XTW, (ch * NFREE) % XTW + NFREE)
                xap = xt[:, xcols]

                # ---------- layer 0: project + square ----------
                v0sq = []
                for m in range(NC0):
                    vps = vps_pool.tile([128, NFREE], dt.float32, tag="vps")
                    for h in range(NFREE // NMM):
                        hs = slice(h * NMM, (h + 1) * NMM)
                        nc.tensor.matmul(
                            vps[:, hs],
                            p0_sb[:, m * 128:(m + 1) * 128],
                            xap[:, hs],
                            start=True, stop=True,
                        )
                    vsq = sq_pool.tile([128, NFREE], dt.bfloat16, tag="vsq")
                    square(m, vsq[:], vps[:], sq_pool)
                    v0sq.append(vsq)

                # ---------- layer 0: contract ----------
                h0ps = h0_pool.tile([S0, NFREE], dt.float32, tag="h0")
                for m in range(NC0):
                    for h in range(NFREE // NMM):
                        hs = slice(h * NMM, (h + 1) * NMM)
                        nc.tensor.matmul(
                            h0ps[:, hs],
                            c0_sb[:, m * S0:(m + 1) * S0],
                            v0sq[m][:, hs],
                            start=(m == 0), stop=(m == NC0 - 1),
                        )

                # ---------- split h0: next-hidden (relu, bf16) + direct0 ----------
                u = u_pool.tile([KU, NFREE], dt.bfloat16, tag="u")
                nc.vector.tensor_copy(out=u[NHF:KU, :], in_=xap)
                nc.scalar.activation(u[0:NHF, :], h0ps[0:NHF, :], AF.Relu,
                                     bias=b0_sb[0:NHF, 0:1], scale=1.0)
                d0 = d_pool.tile([S0 - NHF, NBPC, D], dt.float32, tag="d0")
                nc.scalar.activation(d0[:], h0ps[NHF:S0, :], AF.Relu,
                                     bias=b0_sb[NHF:S0, 0:1], scale=1.0)
                nc.vector.tensor_reduce(
                    out=out0_sb[:, ch * NBPC:(ch + 1) * NBPC],
                    in_=d0[:], axis=mybir.AxisListType.X, op=ALU.add,
                )

                # ---------- layer 1: project + square ----------
                usq = u_pool.tile([KU, NFREE], dt.bfloat16, tag="usq")
                nc.vector.tensor_mul(usq[:], u[:], u[:])
                v1sq = []
                for m in range(NC1):
                    vps = vps_pool.tile([128, NFREE], dt.float32, tag="vps")
                    for h in range(NFREE // NMM):
                        hs = slice(h * NMM, (h + 1) * NMM)
                        nc.tensor.matmul(
                            vps[:, hs],
                            p1_sb[:, m * 128:(m + 1) * 128],
                            u[:, hs],
                            start=True, stop=True,
                        )
                    vsq = sq_pool.tile([128, NFREE], dt.bfloat16, tag="vsq")
                    square(m + 8, vsq[:], vps[:], sq_pool)
                    v1sq.append(vsq)

                # ---------- layer 1: contract (20 pair chunks + singles) ----------
                h1ps = h1_pool.tile([S1, NFREE], dt.float32, tag="h1")
                for m in range(NC1):
                    for h in range(NFREE // NMM):
                        hs = slice(h * NMM, (h + 1) * NMM)
                        nc.tensor.matmul(
                            h1ps[:, hs],
                            c1p_sb[:, m * S1:(m + 1) * S1],
                            v1sq[m][:, hs],
                            start=(m == 0), stop=False,
                        )
                for h in range(NFREE // NMM):
                    hs = slice(h * NMM, (h + 1) * NMM)
                    nc.tensor.matmul(
                        h1ps[:, hs], c1s_sb[:], usq[:, hs],
                        start=False, stop=True,
                    )

                d1 = d_pool.tile([S1, NBPC, D], dt.float32, tag="d1")
                nc.scalar.activation(d1[:], h1ps[:], AF.Relu,
                                     bias=b1_sb[:, 0:1], scale=1.0)
                nc.vector.tensor_reduce(
                    out=out1_sb[:, ch * NBPC:(ch + 1) * NBPC],
                    in_=d1[:], axis=mybir.AxisListType.X, op=ALU.add,
                )

            nc.gpsimd.dma_start(out=out_d[0:S0 - NHF, :], in_=out0_sb[:])
            nc.gpsimd.dma_start(out=out_d[S0 - NHF:, :], in_=out1_sb[:])

    _split_multi_waits(nc)
    return nc


def kernel(x, W0, b0, W1, b1):
    from concourse.bass_utils import run_bass_kernel_spmd

    x = np.asarray(x)
    w = _build_host_weights(W0, b0, W1, b1)

    if "nc" not in _cached:
        _cached["nc"] = _build_nc()
    nc = _cached["nc"]

    NXT = 4
    XTW = BD // NXT
    in_maps = []
    for c in range(NCORES):
        xs = x[c * BC:(c + 1) * BC]                        # [256, 40, 32]
        xtc = np.ascontiguousarray(
            xs.transpose(1, 0, 2).reshape(F0, BD)
        ).astype(BF16)                                     # [40, 8192]
        m = {f"xt{t}": np.ascontiguousarray(xtc[:, t * XTW:(t + 1) * XTW])
             for t in range(NXT)}
        m.update(w)
        in_maps.append(m)

    import os
    trace = bool(os.environ.get("CIN_TRACE"))
    res = run_bass_kernel_spmd(nc, in_maps, list(range(NCORES)), trace=trace)
    _cached["last_res"] = res
    outs = []
    for c in range(NCORES):
        o = res.results[c]["out"]                          # [192, 256]
        outs.append(np.ascontiguousarray(o.T))             # [256, 192]
    return np.concatenate(outs, axis=0).astype(np.float32)


# revision 16
# speedup vs baseline: 1.5536x; 1.5536x over previous
"""CIN (Compressed Interaction Network) kernel for Trainium2, 8 NeuronCores.

Reference computation (per batch b, embedding dim d; x has 40 field vectors):
  h0[s] = relu( sum_{i,j} x_i x_j W0[i,j,s] + b0[s] )          s in 0..128
  nh    = h0[0:64];  d0 = h0[64:128]
  h1[s] = relu( sum_{i<40, j<64} x_i nh_j W1[i,j,s] + b1[s] )
  out   = concat(d0, h1, axis=s) summed over d                 -> (B, 192)

Strategy
--------
Pure data parallel over the batch (B=2048 -> 256 per core).  Per (b, d)
pair ("bd" column, 8192 per core) the two contractions are quadratic /
bilinear forms over small vectors.  Using the polarization identity
  a*b = ((a+b)^2 - a^2 - b^2) / 2
each layer becomes:   project (PE matmul, K-major)  ->  square
(ACT/DVE elementwise, PSUM->SBUF)  ->  contract (PE matmul, accumulate).
This avoids all transposes and partition broadcasts; every matmul is
K-major with bd on the free axis.  All matmul operands are bf16 (fp32
accumulation in PSUM); coefficient matrices are precomputed on host.

Layer 0: features = 780 pair sums (x_i+x_j, i<j) + 40 singles (x_i),
padded to 896 = 7*128.  Layer 1: 2560 pair sums (x_i + nh_j) in 20
chunks + one K=104 contraction against squared [x; nh] singles.
"""

import numpy as np
import ml_dtypes

B, F0, D = 2048, 40, 32
NCORES = 8
BC = B // NCORES       # 256 batches per core
BD = BC * D            # 8192 bd columns per core
NHF = 64               # next-hidden fields (split_half of 128)
S0 = 128               # layer 0 outputs
S1 = 128               # layer 1 outputs
KU = F0 + NHF          # 104

PAIRS0 = [(i, j) for i in range(F0) for j in range(i + 1, F0)]   # 780
R0 = len(PAIRS0) + F0   # 820
NC0 = 7                 # L0 feature chunks (896 padded)
R0P = NC0 * 128
R1 = F0 * NHF           # 2560 L1 pair features
NC1 = R1 // 128         # 20

NFREE = 1024            # bd columns per pipeline chunk
NMM = 512               # max free dim per matmul instruction
NCHUNKS = BD // NFREE
NBPC = NFREE // D       # batches per chunk

BF16 = ml_dtypes.bfloat16

_cached = {}


def _build_host_weights(W0, b0, W1, b1):
    """Precompute projection/coefficient matrices (float64 for the
    cancellation-prone single coefficients, cast to bf16 at the end)."""
    W0 = np.asarray(W0, np.float64)
    W1 = np.asarray(W1, np.float64)

    # ---- layer 0 ----
    p0 = np.zeros((F0, R0P), np.float64)
    c0 = np.zeros((R0P, S0), np.float64)
    Ssym = (W0 + W0.transpose(1, 0, 2)) / 2.0          # [i, j, s]
    for k, (i, j) in enumerate(PAIRS0):
        p0[i, k] = 1.0
        p0[j, k] = 1.0
        c0[k] = Ssym[i, j]
    for i in range(F0):
        k = len(PAIRS0) + i
        p0[i, k] = 1.0
        c0[k] = W0[i, i] - (Ssym[i].sum(axis=0) - Ssym[i, i])
    # chunk layout: feature f lives at partition f%128, free col (f//128)*S0 + s
    c0_sb = c0.reshape(NC0, 128, S0).transpose(1, 0, 2).reshape(128, NC0 * S0)

    # ---- layer 1 ----
    # U layout (SBUF partition rows must start 32-aligned): nh_j at row j
    # (0:64), x_i at row 64+i (64:104).
    p1 = np.zeros((KU, R1), np.float64)
    c1p = np.zeros((R1, S1), np.float64)
    for i in range(F0):
        for j in range(NHF):
            k = i * NHF + j
            p1[NHF + i, k] = 1.0
            p1[j, k] = 1.0
            c1p[k] = W1[i, j] / 2.0
    c1p_sb = c1p.reshape(NC1, 128, S1).transpose(1, 0, 2).reshape(128, NC1 * S1)
    c1s = np.zeros((KU, S1), np.float64)
    c1s[:NHF] = -0.5 * W1.sum(axis=0)                  # vs nh_j^2
    c1s[NHF:] = -0.5 * W1.sum(axis=1)                  # vs x_i^2

    return {
        "p0": p0.astype(BF16),
        "c0": c0_sb.astype(BF16),
        "p1": p1.astype(BF16),
        "c1p": c1p_sb.astype(BF16),
        "c1s": c1s.astype(BF16),
        "b0": np.asarray(b0, np.float32).reshape(S0, 1),
        "b1": np.asarray(b1, np.float32).reshape(S1, 1),
    }


def _split_multi_waits(nc):
    """The walrus build in this container rejects any instruction carrying
    more than one sync wait ("Too many sync wait commands").  Hoist all but
    one wait of every multi-wait instruction onto same-engine NOPs placed
    immediately before it (engines execute their stream in order, so this
    preserves the happens-before edges)."""
    import concourse.mybir as mybir

    n = 0
    for blk in nc.main_func.blocks:
        insts = blk.instructions
        out = []
        changed = False
        for inst in insts:
            si = getattr(inst, "sync_info", None)
            if si is not None and si.on_wait and len(si.on_wait) > 1:
                waits = list(si.on_wait)
                for w in waits[:-1]:
                    nop = mybir.InstNoOp(
                        name=f"waitsplit_{n}",
                        engine=inst.engine,
                        sync_info=mybir.SyncInfo(on_wait=[w], on_update=[]),
                        bass_nofuse=True,
                    )
                    n += 1
                    out.append(nop)
                si.on_wait = waits[-1:]
                changed = True
            out.append(inst)
        if changed:
            blk.instructions = out
    return n


def _build_nc():
    import concourse.bass as bass
    import concourse.tile as tile
    import concourse.mybir as mybir

    dt = mybir.dt
    AF = mybir.ActivationFunctionType
    ALU = mybir.AluOpType

    nc = bass.Bass()

    NXT = 4                       # x input split into NXT tiles for DMA overlap
    XTW = BD // NXT               # 2048 cols each
    xt_d = [
        nc.dram_tensor(f"xt{t}", [F0, XTW], dt.bfloat16, kind="ExternalInput")
        for t in range(NXT)
    ]
    p0_d = nc.dram_tensor("p0", [F0, R0P], dt.bfloat16, kind="ExternalInput")
    c0_d = nc.dram_tensor("c0", [128, NC0 * S0], dt.bfloat16, kind="ExternalInput")
    p1_d = nc.dram_tensor("p1", [KU, R1], dt.bfloat16, kind="ExternalInput")
    c1p_d = nc.dram_tensor("c1p", [128, NC1 * S1], dt.bfloat16, kind="ExternalInput")
    c1s_d = nc.dram_tensor("c1s", [KU, S1], dt.bfloat16, kind="ExternalInput")
    b0_d = nc.dram_tensor("b0", [S0, 1], dt.float32, kind="ExternalInput")
    b1_d = nc.dram_tensor("b1", [S1, 1], dt.float32, kind="ExternalInput")
    out_d = nc.dram_tensor("out", [S0 - NHF + S1, BC], dt.float32,
                           kind="ExternalOutput")

    with tile.TileContext(nc) as tc:
        with (
            tc.tile_pool(name="const", bufs=1) as const_pool,
            tc.tile_pool(name="xt", bufs=1) as xt_pool,
            tc.tile_pool(name="sq", bufs=12) as sq_pool,
            tc.tile_pool(name="u", bufs=3) as u_pool,
            tc.tile_pool(name="d", bufs=2) as d_pool,
            tc.tile_pool(name="outp", bufs=1) as out_pool,
            tc.tile_pool(name="vps", bufs=3, space="PSUM") as vps_pool,
            tc.tile_pool(name="hps", bufs=1, space="PSUM") as h_pool,
        ):
            p0_sb = const_pool.tile([F0, R0P], dt.bfloat16)
            c0_sb = const_pool.tile([128, NC0 * S0], dt.bfloat16)
            p1_sb = const_pool.tile([KU, R1], dt.bfloat16)
            c1p_sb = const_pool.tile([128, NC1 * S1], dt.bfloat16)
            c1s_sb = const_pool.tile([KU, S1], dt.bfloat16)
            b0_sb = const_pool.tile([S0, 1], dt.float32)
            b1_sb = const_pool.tile([S1, 1], dt.float32)
            nc.gpsimd.dma_start(out=p0_sb[:], in_=p0_d[:])
            nc.gpsimd.dma_start(out=c0_sb[:], in_=c0_d[:])
            nc.gpsimd.dma_start(out=p1_sb[:], in_=p1_d[:])
            nc.gpsimd.dma_start(out=c1p_sb[:], in_=c1p_d[:])
            nc.gpsimd.dma_start(out=c1s_sb[:], in_=c1s_d[:])
            nc.gpsimd.dma_start(out=b0_sb[:], in_=b0_d[:])
            nc.gpsimd.dma_start(out=b1_sb[:], in_=b1_d[:])

            xt_sb = []
            for t in range(NXT):
                xt = xt_pool.tile([F0, XTW], dt.bfloat16, tag=f"xt{t}")
                nc.gpsimd.dma_start(out=xt[:], in_=xt_d[t][:])
                xt_sb.append(xt)

            out0_sb = out_pool.tile([S0 - NHF, BC], dt.float32, tag="o0")
            out1_sb = out_pool.tile([S1, BC], dt.float32, tag="o1")

            NH2 = NFREE // NMM   # matmul halves per chunk column block

            # Even spread of chain-squares (DVE cast + GpSimd mul) among the
            # 27 per-chunk PSUM squares; the rest go to ACT directly.
            NSQ = NC0 + NC1
            N_CHAIN = 10
            CHAIN_SET = {m for m in range(NSQ)
                         if (m * N_CHAIN) // NSQ != ((m + 1) * N_CHAIN) // NSQ}

            def square(idx, dst, src):
                # PSUM evacuation split: ACT squares directly; the rest go
                # DVE copy (PSUM->SBUF bf16) + GpSimd multiply (SBUF bf16).
                # DVE cannot read the PSUM operand twice (NCC_IBVF027).
                if idx not in CHAIN_SET:
                    nc.scalar.square(dst, src)
                else:
                    tmp = sq_pool.tile(list(dst.shape), dst.dtype,
                                       tag="sqtmp", bufs=6)
                    nc.vector.tensor_copy(out=tmp[:], in_=src)
                    nc.gpsimd.tensor_mul(dst, tmp[:], tmp[:])

            def xap_of(ch):
                xt = xt_sb[(ch * NFREE) // XTW]
                c0_ = (ch * NFREE) % XTW
                return xt[:, c0_:c0_ + NFREE]

            def mm_s1l0(ch, m, vps):
                for h in range(NH2):
                    hs = slice(h * NMM, (h + 1) * NMM)
                    nc.tensor.matmul(
                        vps[:, hs], p0_sb[:, m * 128:(m + 1) * 128],
                        xap_of(ch)[:, hs], start=True, stop=True,
                    )

            def mm_s3l0(ch, m, h0ps, v0sq):
                for h in range(NH2):
                    hs = slice(h * NMM, (h + 1) * NMM)
                    nc.tensor.matmul(
                        h0ps[:, hs], c0_sb[:, m * S0:(m + 1) * S0],
                        v0sq[m][:, hs], start=(m == 0), stop=(m == NC0 - 1),
                    )

            def mm_s1l1(ch, m, vps, u):
                for h in range(NH2):
                    hs = slice(h * NMM, (h + 1) * NMM)
                    nc.tensor.matmul(
                        vps[:, hs], p1_sb[:, m * 128:(m + 1) * 128],
                        u[:, hs], start=True, stop=True,
                    )

            def mm_s3l1(ch, m, h1ps, v1sq, usq):
                for h in range(NH2):
                    hs = slice(h * NMM, (h + 1) * NMM)
                    if m < NC1:
                        nc.tensor.matmul(
                            h1ps[:, hs], c1p_sb[:, m * S1:(m + 1) * S1],
                            v1sq[m][:, hs], start=(m == 0), stop=False,
                        )
                    else:
                        nc.tensor.matmul(
                            h1ps[:, hs], c1s_sb[:], usq[:, hs],
                            start=False, stop=True,
                        )

            # Per-chunk live state, keyed by chunk index.
            st = {}

            for i in range(NCHUNKS + 2):
                cA = i          # S1L0 of chunk i
                cB = i - 1      # S3L0 + post0 + S1L1 of chunk i-1
                cE = i - 2      # S3L1 + post1 of chunk i-2

                if cA < NCHUNKS:
                    st[cA] = {"v0sq": [], "v1sq": []}

                # ---- step 1: S3L0(cB) interleaved with S1L0(cA) ----
                for m in range(NC0):
                    if 0 <= cB < NCHUNKS:
                        if m == 0:
                            st[cB]["h0ps"] = h_pool.tile(
                                [S0, NFREE], dt.float32, tag="hps",
                                name=f"h0ps_{cB}")
                        mm_s3l0(cB, m, st[cB]["h0ps"], st[cB]["v0sq"])
                    if cA < NCHUNKS:
                        vps = vps_pool.tile([128, NFREE], dt.float32, tag="vps")
                        mm_s1l0(cA, m, vps)
                        vsq = sq_pool.tile([128, NFREE], dt.bfloat16,
                                           tag="v0sq", bufs=12)
                        square(m, vsq[:], vps[:])
                        st[cA]["v0sq"].append(vsq)

                # ---- step 2: post0(cB): relu->u, x copy, d0 relu+reduce ----
                if 0 <= cB < NCHUNKS:
                    h0ps = st[cB]["h0ps"]
                    u = u_pool.tile([KU, NFREE], dt.bfloat16, tag="u")
                    nc.vector.tensor_copy(out=u[NHF:KU, :], in_=xap_of(cB))
                    nc.scalar.activation(u[0:NHF, :], h0ps[0:NHF, :], AF.Relu,
                                         bias=b0_sb[0:NHF, 0:1], scale=1.0)
                    d0 = d_pool.tile([S0 - NHF, NBPC, D], dt.float32, tag="d0")
                    nc.scalar.activation(d0[:], h0ps[NHF:S0, :], AF.Relu,
                                         bias=b0_sb[NHF:S0, 0:1], scale=1.0)
                    nc.vector.tensor_reduce(
                        out=out0_sb[:, cB * NBPC:(cB + 1) * NBPC],
                        in_=d0[:], axis=mybir.AxisListType.X, op=ALU.add,
                    )
                    usq = u_pool.tile([KU, NFREE], dt.bfloat16, tag="usq")
                    nc.vector.tensor_mul(usq[:], u[:], u[:])
                    st[cB]["u"] = u
                    st[cB]["usq"] = usq

                # ---- step 3: S3L1(cE) interleaved with S1L1(cB) ----
                e_list = list(range(NC1 + 1)) if 0 <= cE else []
                d_list = list(range(NC1)) if 0 <= cB < NCHUNKS else []
                ei = 0
                if e_list:
                    st[cE]["h1ps"] = h_pool.tile([S1, NFREE], dt.float32,
                                                 tag="hps",
                                                 name=f"h1ps_{cE}")
                    for _ in range(2):
                        if ei < len(e_list):
                            mm_s3l1(cE, e_list[ei], st[cE]["h1ps"],
                                    st[cE]["v1sq"], st[cE]["usq"])
                            ei += 1
                for m in d_list:
                    vps = vps_pool.tile([128, NFREE], dt.float32, tag="vps")
                    mm_s1l1(cB, m, vps, st[cB]["u"])
                    vsq = sq_pool.tile([128, NFREE], dt.bfloat16,
                                       tag="v1sq", bufs=32)
                    square(NC0 + m, vsq[:], vps[:])
                    st[cB]["v1sq"].append(vsq)
                    if e_list and ei < len(e_list):
                        mm_s3l1(cE, e_list[ei], st[cE]["h1ps"],
                                st[cE]["v1sq"], st[cE]["usq"])
                        ei += 1
                while e_list and ei < len(e_list):
                    mm_s3l1(cE, e_list[ei], st[cE]["h1ps"],
                            st[cE]["v1sq"], st[cE]["usq"])
                    ei += 1

                # ---- step 4: post1(cE): relu d1 + reduce ----
                if 0 <= cE:
                    d1 = d_pool.tile([S1, NBPC, D], dt.float32, tag="d1")
                    nc.scalar.activation(d1[:], st[cE]["h1ps"][:], AF.Relu,
                                         bias=b1_sb[:, 0:1], scale=1.0)
                    nc.vector.tensor_reduce(
                        out=out1_sb[:, cE * NBPC:(cE + 1) * NBPC],
                        in_=d1[:], axis=mybir.AxisListType.X, op=ALU.add,
                    )
                    del st[cE]

            nc.gpsimd.dma_start(out=out_d[0:S0 - NHF, :], in_=out0_sb[:])
            nc.gpsimd.dma_start(out=out_d[S0 - NHF:, :], in_=out1_sb[:])

    _split_multi_waits(nc)
    return nc


def kernel(x, W0, b0, W1, b1):
    from concourse.bass_utils import run_bass_kernel_spmd

    x = np.asarray(x)
    w = _build_host_weights(W0, b0, W1, b1)

    if "nc" not in _cached:
        _cached["nc"] = _build_nc()
    nc = _cached["nc"]

    NXT = 4
    XTW = BD // NXT
    in_maps = []
    for c in range(NCORES):
        xs = x[c * BC:(c + 1) * BC]                        # [256, 40, 32]
        xtc = np.ascontiguousarray(
            xs.transpose(1, 0, 2).reshape(F0, BD)
        ).astype(BF16)                                     # [40, 8192]
        m = {f"xt{t}": np.ascontiguousarray(xtc[:, t * XTW:(t + 1) * XTW])
             for t in range(NXT)}
        m.update(w)
        in_maps.append(m)

    import os
    trace = bool(os.environ.get("CIN_TRACE"))
    res = run_bass_kernel_spmd(nc, in_maps, list(range(NCORES)), trace=trace)
    _cached["last_res"] = res
    outs = []
    for c in range(NCORES):
        o = res.results[c]["out"]                          # [192, 256]
        outs.append(np.ascontiguousarray(o.T))             # [256, 192]
    return np.concatenate(outs, axis=0).astype(np.float32)
